# revision 71
# baseline (speedup 1.0000x reference)
"""Deformable Conv2d (3x3, pad=1, stride=1) on Trainium2 — Bass/Tile kernel.

Sharding: data-parallel over batch across 8 NeuronCores (B=8 -> 1 image/core);
weights replicated. Per-core pipeline (all 16-bit data in fp16):
  host prep: x cast to fp16 and pre-padded to the 66-wide grid; a doubled
             pixel-major gather table xq where row r = [x[p] | x[p+W]]
             (p = r-1, zero guards), so ONE 2KB SWDGE descriptor starting
             at row(y0,x0) fetches all 4 bilinear corners; weights
             pre-packed and cast on host.
  per-chunk prep (staggered injection into the previous chunk's tap loop,
  so no in-order engine ever waits on prep inputs):
    offset conv (18ch 3x3) as PSUM-accumulated PE matmuls over contiguous
    windows (PE) -> bias add (ACT) -> row transposes to pixel-major (PE+ACT)
    -> bilinear coords/corner weights U0/U1 + quad-gather row indices (DVE;
    floor() via the 1.5*2^23 magic-add; OOB corners get zero weight,
    matching zero-pad semantics) -> wrapped-index staging (SP DMAs).
  main loop (flattened 4 chunks x 9 taps, gathers prefetched 3 taps ahead,
  matmuls deferred one tap so PE never stalls on the ACT copyback):
    SWDGE quad-gather of 1024 pixels -> [128px, 8, 1024] (4 corner slabs);
    combine per unit-pair: 8 two-scalar tensor_scalar (4x DVE mode;
    scalars wy, wx fold the corner-weight product) + one 1024-wide
    tensor_tensor pair-add (DVE) + one 512-wide final pair-add (Pool);
    PE transposes to channel-major into batched [128,1024] fp16 PSUM
    tiles; single ACT copyback per (tap, cc); main conv as
    PSUM-accumulated fp16 matmuls (contraction = (channel, tap), 36
    steps); fp16 output.
"""
import sys

sys.path.insert(0, "/opt/trn_rl_repo")

import numpy as np

import concourse.mybir as mybir
from concourse import bacc
from concourse import bass_utils
from concourse.tile import TileContext
from concourse.bass_types import AP
from concourse.masks import make_identity

B, C, O, H, W = 8, 256, 256, 64, 64
HW = H * W                  # 4096
NCORES = 8
NCHUNK = 4                  # pixel chunks in the main loop
CH = HW // NCHUNK           # 1024 pixels / chunk
JG = CH // 128              # 8 j-groups of 128 pixels / chunk
NJG = HW // 128             # 32 j-groups over the whole image
CR = H // NCHUNK            # 16 image rows / chunk
W2 = W + 2                  # padded row width (66)
H3 = H + 3                  # padded rows (1 top + 64 + 2 bottom)
NROWS = HW + 68             # xq rows: 65 top guards + 4096 + 3 tail
MAGIC = 12582912.0          # 1.5 * 2^23: float32 round-to-int bias
AluOp = mybir.AluOpType
Copy = mybir.ActivationFunctionType.Copy


def _emit(nc):
    f32, f16, i16 = mybir.dt.float32, mybir.dt.float16, mybir.dt.int16

    x_in = nc.dram_tensor("x", [2, 128, H3 * W2], f16, kind="ExternalInput")
    offw = nc.dram_tensor("offw", [128, 2, 9, 18], f16, kind="ExternalInput")
    offb = nc.dram_tensor("offb", [18, 1], f32, kind="ExternalInput")
    convw = nc.dram_tensor("convw", [128, 18, 256], f16, kind="ExternalInput")
    kgrid_d = nc.dram_tensor("kgrid", [128, NJG, 18], f32,
                             kind="ExternalInput")
    xq = nc.dram_tensor("xq", [NROWS, 512], f16, kind="ExternalInput")
    repl_d = nc.dram_tensor("repl16", [16, 128], mybir.dt.float32,
                            kind="ExternalInput")
    y_out = nc.dram_tensor("y", [O, H * W], f16, kind="ExternalOutput")

    with TileContext(nc) as tc:
        with tc.tile_pool(name="consts", bufs=1) as consts, \
             tc.tile_pool(name="pb", bufs=2) as pb, \
             tc.tile_pool(name="gather", bufs=5) as gp, \
             tc.tile_pool(name="work", bufs=6) as wk, \
             tc.tile_pool(name="skp", bufs=4) as skp, \
             tc.tile_pool(name="outp", bufs=3) as op_pool, \
             tc.tile_pool(name="ps_a", bufs=1, space="PSUM") as ps_a, \
             tc.tile_pool(name="ps_t", bufs=3, space="PSUM") as ps_t, \
             tc.tile_pool(name="ps_acc", bufs=1, space="PSUM") as ps_acc:
            # PSUM banks: accs 4 + pt 3 + pa 1 = 8

            # ---- constants / weights to SBUF ----
            ident = consts.tile([128, 128], f16)
            make_identity(nc, ident)
            ident_f32 = consts.tile([128, 128], f32)
            make_identity(nc, ident_f32)
            offw_sb = consts.tile([128, 2, 9, 18], f16)
            nc.scalar.dma_start(out=offw_sb[:], in_=offw.ap())
            offb_sb = consts.tile([18, 1], f32)
            nc.scalar.dma_start(out=offb_sb[:], in_=offb.ap())
            repl16 = consts.tile([16, 128], f32)
            nc.scalar.dma_start(out=repl16[:], in_=repl_d.ap())

            # host-padded fp16 image on the 66-wide grid; banded load so
            # chunk-0's offset conv starts after the first band lands
            x_pad = consts.tile([128, 2, H3, W2], f16)
            for c in range(NCHUNK):
                r0 = c * CR
                nr = CR + (3 if c == NCHUNK - 1 else 1)
                for cc in range(2):
                    nc.sync.dma_start(
                        out=x_pad[:, cc, r0:r0 + nr, :].rearrange(
                            "c h w -> c (h w)"),
                        in_=x_in.ap()[cc, :, r0 * W2:(r0 + nr) * W2])
            x_flat = x_pad.rearrange("c cc h w -> c cc (h w)")

            convw_sb = consts.tile([128, 18, 256], f16)
            nc.scalar.dma_start(out=convw_sb[:], in_=convw.ap())
            kgrid = consts.tile([128, NJG, 18], f32)
            nc.scalar.dma_start(out=kgrid[:], in_=kgrid_d.ap())

            # ================= per-chunk prep pieces =================
            def phase_a_conv(c, pap=None, ptag="pa"):
                """Offset conv rows [16c,16c+16) -> off66 (PE + ACT bias)."""
                off66 = pb.tile([18, CR * W2], f32, tag="off66", name="off66")
                for t, (r0, rows) in enumerate(((0, 7), (7, 7), (14, 2))):
                    n = rows * W2
                    pa = (pap or ps_a).tile([18, 462], f32, tag=ptag,
                                            name="pa")
                    for k in range(9):
                        ky, kx = k // 3, k % 3
                        base = (c * CR + r0 + ky) * W2 + kx
                        for cc in range(2):
                            nc.tensor.matmul(
                                pa[:, 0:n],
                                offw_sb[:, cc, k, :],
                                x_flat[:, cc, base:base + n],
                                start=(k == 0 and cc == 0),
                                stop=(k == 8 and cc == 1))
                    nc.scalar.activation(
                        off66[:, r0 * W2:r0 * W2 + n], pa[:, 0:n],
                        mybir.ActivationFunctionType.Identity,
                        bias=offb_sb[:, 0:1])
                return off66

            def phase_a_tp(off66):
                """Row transposes -> pixel-major offpx (PE + ACT)."""
                offpx = pb.tile([128, JG, 18], f32, tag="offpx", name="offpx")
                for hl in range(CR):
                    pt = ps_t.tile([64, 18], f32, tag="pt", name="offt")
                    nc.tensor.transpose(
                        pt[:], off66[:, hl * W2:hl * W2 + W],
                        ident_f32[0:18, 0:18])
                    nc.scalar.copy(
                        offpx[(hl % 2) * 64:(hl % 2) * 64 + 64, hl // 2, :],
                        pt[:])
                return offpx

            def phase_b(c, offpx):
                """Corner weights U0/U1 + quad-gather indices (DVE)."""
                shp = [128, JG, 18]
                tl = {n: pb.tile(shp, f32, name=f"{n}_{c}", tag=n)
                      for n in ("PP", "FF", "II", "M0", "M1", "T1")}
                U0 = pb.tile(shp, f32, name=f"U0_{c}", tag=f"U0_{c % 2}",
                             bufs=1)
                U1 = pb.tile(shp, f32, name=f"U1_{c}", tag=f"U1_{c % 2}",
                             bufs=1)
                tb = pb.tile([128, JG, 9], f32, name=f"tb_{c}", tag="tb")
                idx16 = pb.tile([128, 9, JG], f32, name=f"idx16_{c}",
                                tag="idx16")

                def ts(out, in0, s, op):
                    nc.vector.tensor_scalar(out=out, in0=in0, scalar1=s,
                                            scalar2=None, op0=op)

                PP, FF, II = tl["PP"], tl["FF"], tl["II"]
                M0, M1, T1 = tl["M0"], tl["M1"], tl["T1"]
                nc.vector.tensor_add(PP[:], offpx[:],
                                     kgrid[:, c * JG:(c + 1) * JG, :])
                ts(T1[:], PP[:], 0.5, AluOp.subtract)
                ts(T1[:], T1[:], MAGIC, AluOp.add)
                ts(II[:], T1[:], MAGIC, AluOp.subtract)    # II = floor(PP)
                nc.vector.tensor_sub(FF[:], PP[:], II[:])  # frac in [0,1)
                ts(M0[:], II[:], 0.0, AluOp.is_ge)
                ts(T1[:], II[:], 63.0, AluOp.is_le)
                nc.vector.tensor_mul(M0[:], M0[:], T1[:])
                ts(M1[:], II[:], -1.0, AluOp.is_ge)
                ts(T1[:], II[:], 62.0, AluOp.is_le)
                nc.vector.tensor_mul(M1[:], M1[:], T1[:])
                nc.vector.tensor_mul(T1[:], FF[:], M0[:])
                nc.vector.tensor_sub(U0[:], M0[:], T1[:])  # (1-f)*m0
                nc.vector.tensor_mul(U1[:], FF[:], M1[:])  # f*m1
                # negated copy: cy=1 slabs are weighted by -wy1 so the
                # Pool final pair-add becomes a subtract (faster gpsimd op)
                U1n = pb.tile(shp, f32, name=f"U1n_{c}", tag=f"U1n_{c % 2}",
                              bufs=1)
                nc.vector.tensor_scalar(out=U1n[:], in0=U1[:], scalar1=-1.0,
                                        scalar2=None, op0=AluOp.mult)
                # gather row: clamp(clamp(y0,-1,63)*64 + x0, -65, 4095) + 65
                # (y0=-1 fetches top-guard zeros for the dead y0 corner and
                #  the true y=0 row for the y1 corner)
                ts(T1[:, :, 0:18:2], II[:, :, 0:18:2], -1.0, AluOp.max)
                ts(T1[:, :, 0:18:2], T1[:, :, 0:18:2], 63.0, AluOp.min)
                ts(tb[:], T1[:, :, 0:18:2], 64.0, AluOp.mult)
                nc.vector.tensor_add(tb[:], tb[:], II[:, :, 1:18:2])
                ts(tb[:], tb[:], -65.0, AluOp.max)
                ts(tb[:], tb[:], 4095.0, AluOp.min)
                ts(tb[:], tb[:], 65.0, AluOp.add)
                nc.vector.tensor_copy(
                    idx16[:].rearrange("p k j -> p j k"), tb[:])
                return U0, U1, U1n, idx16

            def stage_idx(c, idx16):
                """Wrap indices to (i%16, i//16) + replicate to 128 parts
                (SWDGE reads idx from every 16-partition group); 2 HWDGE
                queues in parallel."""
                idxf = pb.tile([16, 9, 64], f32, name=f"idxf_{c}",
                               tag=f"idxf_{c % 2}", bufs=1)
                idxf4 = idxf.rearrange("p k (j q) -> p k j q", j=JG)
                for qh in range(8):
                    eng = nc.sync if qh % 2 == 0 else nc.scalar
                    eng.dma_start(
                        out=idxf4[:, :, :, qh],
                        in_=idx16[qh * 16:(qh + 1) * 16, :, :])
                # replicate to all 8 16-partition groups (SWDGE reads idx
                # per group) via a PE block-identity matmul — no DMA latency
                idxw = pb.tile([128, 9, 64], i16, name=f"idxw_{c}",
                               tag=f"idxw_{c % 2}", bufs=1)
                idxf_f = idxf.rearrange("p k j -> p (k j)")
                idxw_f = idxw.rearrange("p k j -> p (k j)")
                for hh in range(2):
                    pr = ps_t.tile([128, 288], f32, tag="pt", name="idxrep")
                    nc.tensor.matmul(pr[:], repl16[:],
                                     idxf_f[:, hh * 288:(hh + 1) * 288],
                                     start=True, stop=True)
                    nc.scalar.copy(
                        idxw_f[:, hh * 288:(hh + 1) * 288], pr[:])
                return idxw

            # chunk-0 prep fully upfront (gates the first gathers);
            # its offset conv borrows the idle 3-buffer "pt" PSUM set so
            # the 3 conv tiles pipeline instead of serializing on one bank
            prep = {0: phase_b(0, phase_a_tp(phase_a_conv(0, ps_t, "pt")))}
            idxw_t = {0: stage_idx(0, prep[0][3])}
            off66_t, offpx_t = {}, {}

            # ================= main loop (flattened taps) =================
            xq_win = AP(tensor=xq, offset=0, ap=[[512, NROWS - 1], [1, 1024]])
            gtiles = {}
            next_g = [0]

            def pump_gathers(limit):
                while next_g[0] < NCHUNK * 9 and next_g[0] < limit:
                    gi = next_g[0]
                    c2, k2 = gi // 9, gi % 9
                    if c2 not in idxw_t:
                        return
                    g = gp.tile([128, JG, 1024], f16, tag="g", name="g")
                    nc.gpsimd.dma_gather(
                        out_ap=g[:], in_ap=xq_win,
                        idxs_ap=idxw_t[c2][:, k2, :],
                        num_idxs=CH, num_idxs_reg=CH,
                        elem_size=1024, elem_step=512,
                        transpose=False)
                    gtiles[gi] = g
                    next_g[0] += 1

            accs_t = {}
            pend_mm = [None]   # (c, k, sk) deferred by one tap

            def emit_mm(c, k, sk):
                accs = accs_t[c]
                for cc in range(2):
                    for o in range(2):
                        for sub in range(2):
                            nc.tensor.matmul(
                                accs[o * 2 + sub],
                                convw_sb[:, k * 2 + cc,
                                         o * 128:(o + 1) * 128],
                                sk[:, cc, sub * 512:(sub + 1) * 512],
                                start=(k == 0 and cc == 0),
                                stop=(k == 8 and cc == 1))

            def emit_out(c):
                accs = accs_t.pop(c)
                for o in range(2):
                    ob = op_pool.tile([128, CH], f16, tag=f"ob{o}",
                                      name=f"ob{o}")
                    for sub in range(2):
                        nc.scalar.copy(ob[:, sub * 512:(sub + 1) * 512],
                                       accs[o * 2 + sub][:])
                    nc.sync.dma_start(
                        out=y_out.ap()[o * 128:(o + 1) * 128,
                                       c * CH:(c + 1) * CH],
                        in_=ob[:])

            for ti in range(NCHUNK * 9):
                c, k = ti // 9, ti % 9
                if k == 0:
                    accs_t[c] = [ps_acc.tile([128, 512], f32, tag=f"acc{a}",
                                             name=f"acc{a}")
                                 for a in range(4)]
                pump_gathers(ti + 5)
                # staggered prep injection for chunk c+1
                if c + 1 < NCHUNK:
                    if k == 1:
                        off66_t[c + 1] = phase_a_conv(c + 1)
                    elif k == 2:
                        offpx_t[c + 1] = phase_a_tp(off66_t.pop(c + 1))
                    elif k == 3:
                        prep[c + 1] = phase_b(c + 1, offpx_t.pop(c + 1))
                    elif k == 5:
                        idxw_t[c + 1] = stage_idx(c + 1, prep[c + 1][3])

                U0, U1, U1n, _ = prep[c]
                g = gtiles.pop(ti)
                pts = [ps_t.tile([128, 1024], f16, tag="pt",
                                 name=f"pt{cc}") for cc in range(2)]
                for u in range(4):
                    # two 128-pixel units fused per op for wider DVE ops
                    t2 = wk.tile([128, 2, 1024], f16, tag="t2", name="t2")
                    v2 = wk.tile([128, 2, 512], f16, tag="v2", name="v2")
                    s2 = wk.tile([128, 2, 256], f16, tag="s2", name="s2")
                    for h in range(2):
                        jl = 2 * u + h
                        # slabs (cy,cx): (0,0),(1,0),(0,1),(1,1);
                        # weight = Uy_cy * Ux_cx via two-scalar ts (4x mode)
                        for i, (cy, cx) in enumerate(
                                ((0, 0), (1, 0), (0, 1), (1, 1))):
                            uy = (U0 if cy == 0 else U1n)[:, jl,
                                                           2 * k:2 * k + 1]
                            ux = (U0 if cx == 0 else U1)[:, jl,
                                                         2 * k + 1:2 * k + 2]
                            nc.vector.tensor_scalar(
                                out=t2[:, h, i * 256:(i + 1) * 256],
                                in0=g[:, jl, i * 256:(i + 1) * 256],
                                scalar1=uy, scalar2=ux,
                                op0=AluOp.mult, op1=AluOp.mult)
                    # x-interp for both units: [t0+t2, t1+t3] per unit
                    nc.vector.tensor_tensor(
                        out=v2[:], in0=t2[:, :, 0:512], in1=t2[:, :, 512:1024],
                        op=AluOp.add)
                    # y-interp on Pool: s = vy0 - (-vy1) per unit
                    # (subtract: faster gpsimd ucode path than add)
                    nc.gpsimd.tensor_tensor(
                        out=s2[:], in0=v2[:, :, 0:256], in1=v2[:, :, 256:512],
                        op=AluOp.subtract)
                    for h in range(2):
                        jl = 2 * u + h
                        for cc in range(2):
                            nc.tensor.transpose(
                                pts[cc][:, jl * 128:(jl + 1) * 128],
                                s2[:, h, cc * 128:(cc + 1) * 128], ident[:])
                sk = skp.tile([128, 2, 1024], f16, tag="sk", name="sk")
                for cc in range(2):
                    nc.scalar.copy(sk[:, cc, :], pts[cc][:])
                # matmuls deferred one tap: PE never waits the ACT copyback
                if pend_mm[0] is not None:
                    pc, pk, psk = pend_mm[0]
                    emit_mm(pc, pk, psk)
                    if pk == 8:
                        emit_out(pc)
                pend_mm[0] = (c, k, sk)
            pc, pk, psk = pend_mm[0]
            emit_mm(pc, pk, psk)
            emit_out(pc)
    nc.compile()
    return nc


_CACHE = {}


def _get_nc():
    if "nc" not in _CACHE:
        nc = bacc.Bacc("TRN2", target_bir_lowering=False, debug=False,
                       num_devices=NCORES)
        _CACHE["nc"] = _emit(nc)
    return _CACHE["nc"]


def _host_tables():
    if "kgrid" in _CACHE:
        return _CACHE["kgrid"]
    q = np.arange(128)[:, None, None]
    j = np.arange(NJG)[None, :, None]
    ch = np.arange(18)[None, None, :]
    p = j * 128 + q
    k = ch // 2
    d = ch % 2
    ky, kx = k // 3, k % 3
    grid = np.where(d == 0, p // W + ky - 1, p % W + kx - 1).astype(np.float32)
    _CACHE["kgrid"] = np.ascontiguousarray(grid)
    return _CACHE["kgrid"]


def _pack_weights(offset_w, offset_b, conv_w):
    # offw lhsT: [c, cc, k, j] = offset_w[j, cc*128+c, ky, kx]
    ow = offset_w.reshape(18, 2, 128, 9).transpose(2, 1, 3, 0)
    # convw lhsT: [c, (k,cc) chunk, o] = conv_w[o, cc*128+c, k]
    cw = conv_w.reshape(256, 2, 128, 9).transpose(2, 3, 1, 0)  # c, k, cc, o
    cw = cw.reshape(128, 18, 256)
    ob = offset_b.reshape(18, 1)
    return (np.ascontiguousarray(ow, np.float16),
            np.ascontiguousarray(ob, np.float32),
            np.ascontiguousarray(cw, np.float16))


def make_in_maps(x, offset_w, offset_b, conv_w):
    ow, ob, cw = _pack_weights(np.asarray(offset_w), np.asarray(offset_b),
                               np.asarray(conv_w))
    kg = _host_tables()
    rp = (np.arange(128)[None, :] % 16 ==
          np.arange(16)[:, None]).astype(np.float32)
    x16 = np.asarray(x, np.float32).astype(np.float16)
    maps = []
    for b in range(B):
        xi = x16[b].reshape(C, HW)
        # padded image [2, 128, H3, W2]
        xp = np.zeros((2, 128, H3, W2), np.float16)
        xp[:, :, 1:H + 1, 1:W + 1] = xi.reshape(2, 128, H, W)
        # doubled gather table: row r = [x[r-1] | x[r-1+W]]
        xt = np.ascontiguousarray(xi.T)               # [4096, 256]
        xqt = np.zeros((NROWS, 512), np.float16)
        xqt[65:HW + 65, 0:256] = xt
        xqt[1:HW + 1, 256:512] = xt
        maps.append({
            "x": np.ascontiguousarray(xp.reshape(2, 128, H3 * W2)),
            "offw": ow, "offb": ob, "convw": cw, "kgrid": kg,
            "xq": xqt, "repl16": rp,
        })
    return maps


def kernel(x, offset_w, offset_b, conv_w):
    nc = _get_nc()
    in_maps = make_in_maps(x, offset_w, offset_b, conv_w)
    res = bass_utils.run_bass_kernel_spmd(nc, in_maps,
                                          core_ids=list(range(NCORES)))
    out = np.stack([np.asarray(res.results[b]["y"]).reshape(O, H, W)
                    for b in range(B)])
    return out.astype(np.float32)
